# revision 16
# baseline (speedup 1.0000x reference)
"""Trainium2 Bass kernel for nn_Channel: adaptive max-pool(3) -> 16 depthwise
3x3 convs -> sigmoid-sum channel gate -> leaky(gate*x).

Data-parallel over batch: 32 batches -> 4 per core x 8 cores. Weights/biases
replicated. Self-contained: hardcodes shapes from the problem spec.
"""

import numpy as np

import concourse.bacc as bacc
import concourse.bass as bass
import concourse.tile as tile
from concourse import mybir
from concourse.bass_utils import run_bass_kernel_spmd

AFT = mybir.ActivationFunctionType
ALU = mybir.AluOpType
F32 = mybir.dt.float32

B, C, H, W = 32, 256, 96, 96
N_CORES = 8
B_SH = B // N_CORES          # 4 batches per core
P = 128                      # SBUF partitions
G = C // P                   # 2 channel groups
HW = H * W                   # 9216
K = 16                       # number of depthwise convs
NEG = 0.01                   # leaky relu slope (torch default)


def build(repeat: int = 1, copy_only: bool = False):
    nc = bacc.Bacc(None)
    x = nc.dram_tensor("x", [B_SH, C, H, W], F32, kind="ExternalInput")
    # packed per-channel weights+biases: [p, g, k*9 weights .. k biases]
    wb = nc.dram_tensor("wb", [P, G, K * 9 + K], F32, kind="ExternalInput")
    out = nc.dram_tensor("out", [B_SH, C, H, W], F32, kind="ExternalOutput")

    # channel c = g*128 + p -> partition p of group g
    x2 = x.rearrange("b (g p) h w -> (b g) p (h w)", g=G, p=P)
    o2 = out.rearrange("b (g p) h w -> (b g) p (h w)", g=G, p=P)

    NT = B_SH * G  # 8 tiles per core
    DEPTH = 5      # load-ahead depth (xp pool slots)

    with tile.TileContext(nc) as tc:
        with (
            tc.tile_pool(name="xp", bufs=DEPTH) as xp,
            tc.tile_pool(name="cst", bufs=1) as cst,
            tc.tile_pool(name="sm", bufs=4) as sm,
        ):
            wb_t = cst.tile([P, G, K * 9 + K], F32)
            nc.sync.dma_start(wb_t[:], wb[:])
            # warmup read so the wb DMA wait lands here, not on the first
            # TensorTensor (whose ISA format has too few sync-wait slots)
            warm = cst.tile([P, 1], F32)
            nc.vector.tensor_copy(warm[:], wb_t[:, 0, 0:1])

            # seq[j] = tile index of the j-th unit of work; repeat>1 re-runs
            # the whole pass (for differential HW timing) writing identical
            # bytes to out each pass.
            seq = [i % (B_SH * G) for i in range(B_SH * G * repeat)]
            xts = {}

            def load(j):
                xt = xp.tile([P, HW], F32, tag="xt")
                nc.sync.dma_start(xt[:], x2[seq[j]])
                xts[j] = xt

            def compute_store(j):
                i = seq[j]
                g = i % G
                xt = xts.pop(j)
                if copy_only:
                    nc.sync.dma_start(o2[i], xt[:])
                    return
                # 32x32 block max: view [p, hb, wb, h, w], reduce (h, w)
                xv = xt[:].rearrange(
                    "p (hb h wb w) -> p hb wb h w", hb=3, h=32, wb=3, w=32
                )
                pooled = sm.tile([P, 9], F32, tag="pooled")
                nc.vector.reduce_max(
                    pooled[:].rearrange("p (hb wb) -> p hb wb", hb=3),
                    xv,
                    axis=mybir.AxisListType.XY,
                )

                # conv[p,k] = sum_j pooled[p,j] * wt[p,k,j]  (+ bias)
                prod = sm.tile([P, K, 9], F32, tag="prod")
                pooled_b = pooled[:].unsqueeze(1).broadcast_to([P, K, 9])
                wt_v = wb_t[:, g, 0 : K * 9].rearrange("p (k n) -> p k n", k=K)
                nc.vector.tensor_tensor(prod[:], wt_v, pooled_b, ALU.mult)
                conv = sm.tile([P, K], F32, tag="conv")
                nc.vector.reduce_sum(conv[:], prod[:], axis=mybir.AxisListType.X)
                nc.vector.tensor_add(conv[:], conv[:], wb_t[:, g, K * 9 :])

                # gate = sum_k sigmoid(leaky(conv)); scale = leaky(gate)
                lr = sm.tile([P, K], F32, tag="lr")
                nc.scalar.activation(lr[:], conv[:], AFT.Lrelu, alpha=NEG)
                sig = sm.tile([P, K], F32, tag="sig")
                gate = sm.tile([P, 1], F32, tag="gate")
                nc.scalar.activation(sig[:], lr[:], AFT.Sigmoid, accum_out=gate[:])
                s = sm.tile([P, 1], F32, tag="s")
                nc.scalar.activation(s[:], gate[:], AFT.Lrelu, alpha=NEG)

                # out = leaky(s * x), in place on the big tile
                nc.scalar.activation(xt[:], xt[:], AFT.Lrelu, scale=s[:], alpha=NEG)
                nc.sync.dma_start(o2[i], xt[:])

            # software pipeline: loads run DEPTH tiles ahead of stores
            for j in range(min(DEPTH, len(seq))):
                load(j)
            for j in range(len(seq)):
                compute_store(j)
                if j + DEPTH < len(seq):
                    load(j + DEPTH)
    nc.finalize()
    return nc


def _prep_small(w: np.ndarray, b: np.ndarray):
    # wb[p, g, k*9 + i*3 + j] = w[k, g*128+p, i, j]; wb[p, g, 144+k] = b[k, g*128+p]
    wt = w.transpose(1, 0, 2, 3).reshape(G, P, K * 9).transpose(1, 0, 2)
    bt = b.T.reshape(G, P, K).transpose(1, 0, 2)
    return np.ascontiguousarray(np.concatenate([wt, bt], axis=2))


def run(inputs: dict, trace: bool = False):
    x = np.ascontiguousarray(np.asarray(inputs["x"], dtype=np.float32))
    w = np.asarray(inputs["w"], dtype=np.float32)
    b = np.asarray(inputs["b"], dtype=np.float32)
    wb = _prep_small(w, b)

    nc = build()
    in_maps = [
        {"x": np.ascontiguousarray(x[i * B_SH : (i + 1) * B_SH]), "wb": wb}
        for i in range(N_CORES)
    ]
    res = run_bass_kernel_spmd(nc, in_maps, core_ids=list(range(N_CORES)), trace=trace)
    out = np.concatenate([r["out"] for r in res.results], axis=0)
    return out, res


def kernel(**inputs) -> np.ndarray:
    out, _ = run(inputs, trace=False)
    return out


# revision 18
# speedup vs baseline: 1.3205x; 1.3205x over previous
"""Trainium2 Bass kernel for nn_Channel: adaptive max-pool(3) -> 16 depthwise
3x3 convs -> sigmoid-sum channel gate -> leaky(gate*x).

Data-parallel over batch: 32 batches -> 4 per core x 8 cores. Weights/biases
replicated. Self-contained: hardcodes shapes from the problem spec.
"""

import numpy as np

import concourse.bacc as bacc
import concourse.tile as tile
from concourse import mybir
from concourse.bass_utils import run_bass_kernel_spmd

AFT = mybir.ActivationFunctionType
ALU = mybir.AluOpType
F32 = mybir.dt.float32

B, C, H, W = 32, 256, 96, 96
N_CORES = 8
B_SH = B // N_CORES          # 4 batches per core
P = 128                      # SBUF partitions
G = C // P                   # 2 channel groups
HW = H * W                   # 9216
K = 16                       # number of depthwise convs
NEG = 0.01                   # leaky relu slope (torch default)


def build(repeat: int = 1, copy_only: bool = False):
    nc = bacc.Bacc(None)
    x = nc.dram_tensor("x", [B_SH, C, H, W], F32, kind="ExternalInput")
    # packed per-channel weights+biases: [p, g, k*9 weights .. k biases]
    wb = nc.dram_tensor("wb", [P, G, K * 9 + K], F32, kind="ExternalInput")
    out = nc.dram_tensor("out", [B_SH, C, H, W], F32, kind="ExternalOutput")

    # channel c = g*128 + p -> partition p of group g
    x2 = x.rearrange("b (g p) h w -> (b g) p (h w)", g=G, p=P)
    o2 = out.rearrange("b (g p) h w -> (b g) p (h w)", g=G, p=P)

    DEPTH = 5  # load-ahead depth (xp pool slots); 5x36KB/partition in SBUF

    with tile.TileContext(nc) as tc:
        with (
            tc.tile_pool(name="xp", bufs=DEPTH) as xp,
            tc.tile_pool(name="cst", bufs=1) as cst,
            tc.tile_pool(name="sm", bufs=4) as sm,
        ):
            wb_t = cst.tile([P, G, K * 9 + K], F32)
            nc.sync.dma_start(wb_t[:], wb[:])
            # warmup read so the wb DMA wait lands here, not on the first
            # TensorTensor (whose ISA format has too few sync-wait slots)
            warm = cst.tile([P, 1], F32)
            nc.vector.tensor_copy(warm[:], wb_t[:, 0, 0:1])

            # seq[j] = tile index of the j-th unit of work; repeat>1 re-runs
            # the whole pass (for differential HW timing) writing identical
            # bytes to out each pass.
            seq = [i % (B_SH * G) for i in range(B_SH * G * repeat)]
            xts = {}

            def load(j):
                xt = xp.tile([P, HW], F32, tag="xt")
                nc.sync.dma_start(xt[:], x2[seq[j]])
                xts[j] = xt

            def compute_store(j):
                i = seq[j]
                g = i % G
                xt = xts.pop(j)
                if copy_only:
                    nc.sync.dma_start(o2[i], xt[:])
                    return
                # 32x32 block max: view [p, hb, wb, h, w], reduce (h, w)
                xv = xt[:].rearrange(
                    "p (hb h wb w) -> p hb wb h w", hb=3, h=32, wb=3, w=32
                )
                pooled = sm.tile([P, 9], F32, tag="pooled")
                nc.vector.reduce_max(
                    pooled[:].rearrange("p (hb wb) -> p hb wb", hb=3),
                    xv,
                    axis=mybir.AxisListType.XY,
                )

                # conv[p,k] = sum_j pooled[p,j] * wt[p,k,j]  (+ bias)
                prod = sm.tile([P, K, 9], F32, tag="prod")
                pooled_b = pooled[:].unsqueeze(1).broadcast_to([P, K, 9])
                wt_v = wb_t[:, g, 0 : K * 9].rearrange("p (k n) -> p k n", k=K)
                nc.vector.tensor_tensor(prod[:], wt_v, pooled_b, ALU.mult)
                conv = sm.tile([P, K], F32, tag="conv")
                nc.vector.reduce_sum(conv[:], prod[:], axis=mybir.AxisListType.X)
                nc.vector.tensor_add(conv[:], conv[:], wb_t[:, g, K * 9 :])

                # gate = sum_k sigmoid(leaky(conv)); scale = leaky(gate)
                lr = sm.tile([P, K], F32, tag="lr")
                nc.scalar.activation(lr[:], conv[:], AFT.Lrelu, alpha=NEG)
                sig = sm.tile([P, K], F32, tag="sig")
                gate = sm.tile([P, 1], F32, tag="gate")
                nc.scalar.activation(sig[:], lr[:], AFT.Sigmoid, accum_out=gate[:])
                s = sm.tile([P, 1], F32, tag="s")
                nc.scalar.activation(s[:], gate[:], AFT.Lrelu, alpha=NEG)

                # out = leaky(s * x), in place on the big tile
                nc.scalar.activation(xt[:], xt[:], AFT.Lrelu, scale=s[:], alpha=NEG)
                nc.sync.dma_start(o2[i], xt[:])

            # software pipeline: loads run DEPTH tiles ahead of stores
            for j in range(min(DEPTH, len(seq))):
                load(j)
            for j in range(len(seq)):
                compute_store(j)
                if j + DEPTH < len(seq):
                    load(j + DEPTH)
    nc.finalize()
    return nc


def _prep_small(w: np.ndarray, b: np.ndarray):
    # wb[p, g, k*9 + i*3 + j] = w[k, g*128+p, i, j]; wb[p, g, 144+k] = b[k, g*128+p]
    wt = w.transpose(1, 0, 2, 3).reshape(G, P, K * 9).transpose(1, 0, 2)
    bt = b.T.reshape(G, P, K).transpose(1, 0, 2)
    return np.ascontiguousarray(np.concatenate([wt, bt], axis=2))


def run(inputs: dict, trace: bool = False):
    x = np.ascontiguousarray(np.asarray(inputs["x"], dtype=np.float32))
    w = np.asarray(inputs["w"], dtype=np.float32)
    b = np.asarray(inputs["b"], dtype=np.float32)
    wb = _prep_small(w, b)

    nc = build()
    in_maps = [
        {"x": np.ascontiguousarray(x[i * B_SH : (i + 1) * B_SH]), "wb": wb}
        for i in range(N_CORES)
    ]
    res = run_bass_kernel_spmd(nc, in_maps, core_ids=list(range(N_CORES)), trace=trace)
    out = np.concatenate([r["out"] for r in res.results], axis=0)
    return out, res


def kernel(**inputs) -> np.ndarray:
    out, _ = run(inputs, trace=False)
    return out
